# revision 3
# baseline (speedup 1.0000x reference)
"""v24: v21 + paired b0/b1 transposes: Q/K cast into
[128, t, b, d] fp16 layout; one [128,128] PE transpose per tile t covers both
batches (q_t/k_t become [128, N] with b0 in partitions 0-63, b1 in 64-127;
QK matmuls slice the partition halves -> same shapes as before via row
groups h0/h64). Halves transpose+copy count: 64 -> 32.

(v21: v20 + b0 k-tile-0 loaded via a mini-DMA on the scalar HWDGE ring
(parallel receipt with Q; round 0 otherwise waits the full-K receipt,
second in the serial sync-ring chain). v20: b1 casts spread at queue
positions 14/17/20/23 (one-block placement stalls the in-order DVE
queue on b1 DMA receipts wherever it lands). v18: v14 with a 6-matmul warm-up (3.8us continuous, still > the 3.4us
HAM window; flat loads deliver data by ~10us so the old 10-filler burst
had become the critical path into round 0). Flat loads: partition p holds rows
16p..16p+15 (row-block permutation); softmax is permutation-invariant
over k (V loaded with the same order), and the q permutation is
undone by the output DMA access pattern. (v9: b1 casts deferred (their
position in the in-order DVE queue otherwise stalls round 0 on b1's
input DMA completion).

vs v6: Q/K pre-cast to fp16 on DVE so the PE transposes are single-pass
fp16 (fp32 transposes run as 2 matmul passes); epilogue work is woven
into the next half's rounds as queued jobs instead of filler-padded
blocks; minimal warm-up burst.
"""

import os
from collections import deque

import numpy as np

import concourse.bacc as bacc
import concourse.mybir as mybir
import concourse.tile as tile
from concourse.bass_utils import run_bass_kernel_spmd
from concourse.masks import make_identity

B, N, D = 16, 2048, 64
NCORES = 8
BPC = B // NCORES
TEMP = 8.0

NT = N // 128
F32 = mybir.dt.float32
F16 = mybir.dt.float16

_RESULTS = None


def attention_tile_kernel(tc):
    nc = tc.nc
    q = nc.declare_dram_parameter("q", [BPC, N, D], F32, isOutput=False)
    k = nc.declare_dram_parameter("k", [BPC, N, D], F32, isOutput=False)
    v = nc.declare_dram_parameter("v", [BPC, N, D], F32, isOutput=False)
    out = nc.declare_dram_parameter("out", [BPC, N, D], F32, isOutput=True)

    with (
        tc.tile_pool(name="const", bufs=1) as cpool,
        tc.tile_pool(name="inp", bufs=2) as inp,
        tc.tile_pool(name="qkt", bufs=2) as qkt,
        tc.tile_pool(name="exp", bufs=4) as epool,
        tc.tile_pool(name="outs", bufs=2) as outp,
        tc.tile_pool(name="spsum", bufs=2, space="PSUM") as spool,
        tc.tile_pool(name="opsum", bufs=2, space="PSUM") as opool,
        tc.tile_pool(name="tpsum", bufs=2, space="PSUM") as tpool,
    ):
        ident = cpool.tile([128, 128], F32)
        make_identity(nc, ident)
        ident16 = cpool.tile([128, 128], F16)
        nc.vector.tensor_copy(ident16, ident)
        wsrc = cpool.tile([128, 512], F32)
        nc.vector.memset(wsrc, 0.0)
        warm_w = cpool.tile([64, 128], F16)
        nc.vector.tensor_copy(warm_w, wsrc[0:64, 0:128])
        warm_r = cpool.tile([64, 512], F16)
        nc.vector.tensor_copy(warm_r, wsrc[0:64, :])
        warm_sink = cpool.tile([128, 1], F32)

        def tptile(name):
            # one PSUM bank, viewed as fp16 [128,1024] or f32 [128,512]
            return tpool.tile([128, 1024], F16, tag="tp", name=name)

        def filler(tp):
            nc.tensor.matmul(
                tp.bitcast(F32), warm_w, warm_r, start=True, stop=True
            )

        # ---- input loads, both batches (b1's DVE casts deferred) ----
        # paired f16 layout [128, t, b, d]: one [128,128] transpose per tile
        # covers both batches; q_t/k_t hold b0 in partitions 0-63, b1 in
        # 64-127 (QK matmuls slice partition halves; row groups h0/h64)
        vp, qns, kns = [], [], []
        qf = inp.tile([128, NT, BPC, D], F16, tag="qf16", name="qf")
        kf = inp.tile([128, NT, BPC, D], F16, tag="kf16", name="kf")
        q_tp = qkt.tile([128, N], F16, tag="qt", name="qtp")
        k_tp = qkt.tile([128, N], F16, tag="kt", name="ktp")
        q_t = [q_tp[0:64, :], q_tp[64:128, :]]
        k_t = [k_tp[0:64, :], k_tp[64:128, :]]
        cast_later = []
        for b in range(BPC):
            qn = inp.tile([128, NT, D], F32, tag="qnat", name=f"qn{b}")
            kn = inp.tile([128, NT, D], F32, tag="knat", name=f"kn{b}")
            vpb = inp.tile([128, NT, D + 1], F16, tag="vp", name=f"vp{b}")
            vp.append(vpb)
            qns.append(qn)
            kns.append(kn)
        qv2 = [q[b].rearrange("(p r) d -> p r d", p=128) for b in range(BPC)]
        kv2 = [k[b].rearrange("(p r) d -> p r d", p=128) for b in range(BPC)]
        # flat loads, split so round-0's needs (q tiles 0-7 both b, k tile 0
        # both b) arrive first; b0 on the sync ring, b1 on the scalar ring.
        nc.scalar.dma_start(out=kns[0][:, 0:1, :], in_=kv2[0][:, 0:1, :])
        nc.scalar.dma_start(out=kns[1][:, 0:1, :], in_=kv2[1][:, 0:1, :])
        nc.sync.dma_start(out=qns[0][:, 0:8, :], in_=qv2[0][:, 0:8, :])
        nc.scalar.dma_start(out=qns[1][:, 0:8, :], in_=qv2[1][:, 0:8, :])
        nc.sync.dma_start(out=kns[0][:, 1:9, :], in_=kv2[0][:, 1:9, :])
        nc.scalar.dma_start(out=kns[1][:, 1:9, :], in_=kv2[1][:, 1:9, :])
        nc.sync.dma_start(out=qns[0][:, 8:NT, :], in_=qv2[0][:, 8:NT, :])
        nc.scalar.dma_start(out=qns[1][:, 8:NT, :], in_=qv2[1][:, 8:NT, :])
        nc.sync.dma_start(out=kns[0][:, 9:NT, :], in_=kv2[0][:, 9:NT, :])
        nc.scalar.dma_start(out=kns[1][:, 9:NT, :], in_=kv2[1][:, 9:NT, :])
        for b in range(BPC):
            nc.gpsimd.dma_start(
                out=vp[b][:, :, 0:D], in_=v[b].rearrange("(p r) d -> p r d", p=128)
            )
            ones16 = inp.tile([128, NT], F32, tag="ones16", name=f"on{b}")
            nc.vector.memset(ones16, 1.0)
            nc.vector.tensor_copy(vp[b][:, :, D], ones16)
        # early casts: ONLY round-0 needs; the rest is woven so the in-order
        # DVE queue never blocks the first transposes
        nc.vector.tensor_copy(kf[:, 0:1, 0, :], kns[0][:, 0:1, :])
        nc.vector.tensor_copy(kf[:, 0:1, 1, :], kns[1][:, 0:1, :])
        nc.vector.tensor_copy(qf[:, 0:8, 0, :], qns[0][:, 0:8, :])
        nc.vector.tensor_copy(qf[:, 0:8, 1, :], qns[1][:, 0:8, :])

        def transpose_job(nat, tmat, t):
            tp = tptile(f"tp{nc.next_id()}")
            nc.tensor.transpose(tp[:, 0:128], nat[:, t, :, :], ident16)
            nc.vector.tensor_copy(tmat[:, t * 128 : (t + 1) * 128], tp[:, 0:128])

        def cast_one(dst, srcv):
            return lambda: nc.vector.tensor_copy(dst, srcv)

        b1_cast_jobs = []

        jobs = deque()
        for t in range(8):
            jobs.append((qf, q_tp, t))
        jobs.append((kf, k_tp, 0))
        jobs.append(cast_one(kf[:, 1:9, 0, :], kns[0][:, 1:9, :]))
        jobs.append(cast_one(kf[:, 1:9, 1, :], kns[1][:, 1:9, :]))
        for t in range(1, 9):
            jobs.append((kf, k_tp, t))
        jobs.append(cast_one(kf[:, 9:NT, 0, :], kns[0][:, 9:NT, :]))
        jobs.append(cast_one(kf[:, 9:NT, 1, :], kns[1][:, 9:NT, :]))
        for t in range(9, NT):
            jobs.append((kf, k_tp, t))
        jobs.append(cast_one(qf[:, 8:NT, 0, :], qns[0][:, 8:NT, :]))
        jobs.append(cast_one(qf[:, 8:NT, 1, :], qns[1][:, 8:NT, :]))
        for t in range(8, NT):
            jobs.append((qf, q_tp, t))

        def run_job(j):
            if callable(j):
                j()
            else:
                transpose_job(*j)

        # warm-up burst + b0's first-needed transposes
        wps = tptile("wps")
        for _ in range(6):
            filler(wps)
        nc.vector.tensor_copy(warm_sink, wps.bitcast(F32)[:, 0:1])
        for _ in range(9):  # b0: q0..7, k0
            run_job(jobs.popleft())

        # ---- rounds ----
        for b in range(BPC):
            for h in range(2):
                qoff = h * 1024
                out_ps = [
                    opool.tile([D + 1, 512], F32, tag="ops", name=f"o{b}{h}{c}")
                    for c in range(2)
                ]
                pending = []

                def emit_out(pend, out_ps=out_ps, b=b):
                    for kb, e in pend:
                        for c in range(2):
                            nc.tensor.matmul(
                                out_ps[c],
                                vp[b][:, kb, :],
                                e[:, c * 512 : (c + 1) * 512],
                                start=(kb == 0),
                                stop=(kb == NT - 1),
                            )

                for kb in range(NT):
                    s = spool.tile([128, 1024], F32, tag="s")
                    lhs = k_t[b][:, kb * 128 : (kb + 1) * 128]
                    # (q_t[b]/k_t[b] are partition-half slices of the paired tiles)
                    for c in range(2):
                        nc.tensor.matmul(
                            s[:, c * 512 : (c + 1) * 512],
                            lhs,
                            q_t[b][:, qoff + c * 512 : qoff + (c + 1) * 512],
                            start=True,
                            stop=True,
                        )
                    npop = 4 if (b == 0 and h == 0 and kb == 0) else 2
                    for _ in range(npop):
                        if jobs:
                            run_job(jobs.popleft())
                    emit_out(pending)
                    pending = []
                    e = epool.tile([128, 1024], F16, tag="e")
                    nc.scalar.activation(
                        out=e,
                        in_=s,
                        func=mybir.ActivationFunctionType.Exp,
                        scale=1.0 / TEMP,
                    )
                    pending.append((kb, e))
                emit_out(pending)

                # epilogue: copy PSUM accumulators out now (frees opool),
                # queue the normalize+transpose work into later rounds
                last_half = (b == BPC - 1 and h == 1)
                if not last_half:
                    for c in range(2):
                        qc = h * 2 + c
                        osb = outp.tile([D + 1, 512], F32, tag="osb", name=f"os{b}{qc}")
                        nc.vector.tensor_copy(osb, out_ps[c])
                        fout = outp.tile([128, 4, D], F32, tag="fout", name=f"fo{b}{qc}")

                        def ep_job(j, osb=osb, fout=fout):
                            tp = tptile(f"ep{nc.next_id()}")
                            tpf = tp.bitcast(F32)
                            nc.tensor.transpose(
                                tpf[:, 0 : D + 1],
                                osb[:, j * 128 : (j + 1) * 128],
                                ident[0 : D + 1, 0 : D + 1],
                            )
                            rcp = outp.tile([128, 1], F32, tag="rcp")
                            nc.vector.reciprocal(rcp, tpf[:, D : D + 1])
                            nc.vector.tensor_scalar_mul(
                                out=fout[:, j, :], in0=tpf[:, 0:D], scalar1=rcp
                            )

                        def dma_job(b=b, qc=qc, fout=fout):
                            nc.sync.dma_start(
                                out=out[b].rearrange("(p r) d -> p r d", p=128)[
                                    :, 4 * qc : 4 * qc + 4, :
                                ],
                                in_=fout,
                            )

                        for j in range(4):
                            jobs.append((lambda j=j, f=ep_job: f(j)))
                        jobs.append(dma_job)
                else:
                    # final half: nothing left to overlap with -- run a
                    # 4-scratch pipelined epilogue (tpool x2 + retired s
                    # banks x2) in two phase-groups of 4, and put the two
                    # output DMAs on the now-idle scalar + gpsimd rings.
                    osbs, fouts, scr = [], [], []
                    for c in range(2):
                        osb = outp.tile([D + 1, 512], F32, tag="osb", name=f"fos{c}")
                        nc.vector.tensor_copy(osb, out_ps[c])
                        osbs.append(osb)
                        fouts.append(
                            outp.tile([128, 4, D], F32, tag="fout", name=f"ffo{c}")
                        )
                    for i in range(2):
                        scr.append(tptile(f"fep{i}").bitcast(F32)[:, 0:512])
                        scr.append(
                            spool.tile([128, 1024], F32, tag="s", name=f"fsc{i}")[
                                :, 0:512
                            ]
                        )
                    rcps = outp.tile([128, 8], F32, tag="frcp", name="frcp")
                    for g in range(2):
                        for u in range(4 * g, 4 * g + 4):
                            c, j = u // 4, u % 4
                            nc.tensor.transpose(
                                scr[u % 4][:, 0 : D + 1],
                                osbs[c][:, j * 128 : (j + 1) * 128],
                                ident[0 : D + 1, 0 : D + 1],
                            )
                        for u in range(4 * g, 4 * g + 4):
                            nc.vector.reciprocal(
                                rcps[:, u : u + 1], scr[u % 4][:, D : D + 1]
                            )
                        for u in range(4 * g, 4 * g + 4):
                            c, j = u // 4, u % 4
                            nc.vector.tensor_scalar_mul(
                                out=fouts[c][:, j, :],
                                in0=scr[u % 4][:, 0:D],
                                scalar1=rcps[:, u : u + 1],
                            )
                    for c in range(2):
                        qc = h * 2 + c
                        ring = nc.scalar if c == 0 else nc.gpsimd
                        ring.dma_start(
                            out=out[b].rearrange("(p r) d -> p r d", p=128)[
                                :, 4 * qc : 4 * qc + 4, :
                            ],
                            in_=fouts[c],
                        )

        # drain any remaining jobs (last half's epilogue, tail transposes)
        while jobs:
            run_job(jobs.popleft())
    return nc


def build_program():
    nc = bacc.Bacc(None)
    with tile.TileContext(nc) as tc:
        attention_tile_kernel(tc)
    nc.finalize()
    return nc


def kernel(queries: np.ndarray, keys: np.ndarray, values: np.ndarray) -> np.ndarray:
    global _RESULTS
    queries = np.ascontiguousarray(queries, dtype=np.float32)
    keys = np.ascontiguousarray(keys, dtype=np.float32)
    values = np.ascontiguousarray(values, dtype=np.float32)
    nc = build_program()
    in_maps = [
        {
            "q": queries[i * BPC : (i + 1) * BPC],
            "k": keys[i * BPC : (i + 1) * BPC],
            "v": values[i * BPC : (i + 1) * BPC],
        }
        for i in range(NCORES)
    ]
    trace = bool(os.environ.get("ATTN_TRACE"))
    if trace:
        _register_ntff_hook()
    _RESULTS = run_bass_kernel_spmd(nc, in_maps, list(range(NCORES)), trace=trace)
    return np.concatenate([r["out"] for r in _RESULTS.results], axis=0)


def _register_ntff_hook():
    """Dev-only: the slim agent container lacks antenv.axon_hooks; provide it
    so run_bass_kernel_spmd(trace=True) can drive NRT profiling via the axon
    .so directly. No-op unless ATTN_TRACE is set."""
    import sys
    import types

    if "antenv.axon_hooks" in sys.modules:
        return
    try:
        from trn_agent_boot.trn_boot import _ntff_profile_via_ctypes

        h = _ntff_profile_via_ctypes("/opt/axon/libaxon_pjrt.so")
    except Exception:
        return
    mod = types.ModuleType("antenv.axon_hooks")
    mod.get_axon_ntff_profile_hook = lambda: h
    sys.modules["antenv.axon_hooks"] = mod

